# revision 1
# baseline (speedup 1.0000x reference)
"""Trainium2 Bass kernel for the NoisyRNN problem.

Math (reference):
    A = b(Bp-Bp^T) + (1-b)(Bp+Bp^T) - gA*I ; W likewise from Cp
    Z = x @ E_w^T + E_b                        [B, T, 128]
    h_{t+1} = h_t + EPS*(ALPHA*h_t@A + tanh(h_t@W + z_t)),  h_0 = 0
    out = h_T @ D_w^T + D_b                    [B, 10]

Device formulation (per core, batch shard of 64, state transposed [128u, 64b]):
    rescale g = h/EPS:  g_{t+1} = g_t(I + S) + tanh(g_t W' + z_t)
        with S = EPS*ALPHA*A, W' = EPS*W
    split q_{t+1} = tanh(y_t):
        y_t      = g_{t-1} W' + g_{t-1} (S W') + q_t W' + z_t     (PSUM accumulate)
        psum_g_t = g_{t-1} S + q_t                                 (PSUM accumulate, I exact)
        g_t      = g_{t-1} + psum_g_t                              (DVE, f32 master)
        gb_t     = fp16(g_{t-1} + psum_g_t)                        (DVE, fp16 shadow for MM rhs)
    out = (g_{T-1} + psum_g_T) @ (EPS*D_w)^T + D_b

All matmul operands fp16 (PSUM accumulation f32); f32 master state carries the
identity path so precision is preserved (measured end-to-end rel err ~4e-4).
"""

import numpy as np

import concourse.bass as bass
import concourse.tile as tile
from concourse.tile import add_dep_helper
from concourse import bacc, mybir
from concourse.bass_utils import run_bass_kernel_spmd

EPS = 0.01
BETA = 0.8
GAMMA_A = 0.01
GAMMA_W = 0.01
ALPHA = 1.0
NU = 128          # n_units
DIN = 64          # input dim
COUT = 10         # output classes
B_FULL = 512
T_FULL = 1024
NCORES = 8
BL = B_FULL // NCORES  # 64 batch per core

F32 = mybir.dt.float32
F16 = mybir.dt.float16

Tanh = mybir.ActivationFunctionType.Tanh


def build_rnn(T: int, warmup_mms: int = 48, repeats: int = 1,
              n_fill: int = 0) -> bass.Bass:
    """repeats > 1 reruns the recurrence loop over the same x (timing only).

    n_fill: dummy matmuls per step to keep the PE HAM un-throttled.
    """
    nc = bacc.Bacc("TRN2", target_bir_lowering=False, debug=False)

    CHUNK = 64  # timesteps per x DMA chunk

    _last_pe = [None]

    def mm(*args, **kwargs):
        # pin PE program order to emission order (the scheduler otherwise
        # hoists ready matmuls ahead of blocked ones, which drags fresh
        # semaphore waits to the front of the in-order PE stream)
        inst = nc.tensor.matmul(*args, **kwargs)
        cur = getattr(inst, "ins", inst)
        if _last_pe[0] is not None:
            add_dep_helper(cur, _last_pe[0], sync=False, reason="pe-order-pin")
        _last_pe[0] = cur
        return inst

    xw = nc.dram_tensor("xw", [DIN, T * BL], F16, kind="ExternalInput")
    wS = nc.dram_tensor("wS", [NU, NU], F16, kind="ExternalInput")
    wW = nc.dram_tensor("wW", [NU, NU], F16, kind="ExternalInput")
    wM = nc.dram_tensor("wM", [NU, NU], F16, kind="ExternalInput")
    wI = nc.dram_tensor("wI", [NU, NU], F16, kind="ExternalInput")
    wE = nc.dram_tensor("wE", [DIN, NU], F16, kind="ExternalInput")
    wD = nc.dram_tensor("wD", [NU, COUT], F32, kind="ExternalInput")
    bE = nc.dram_tensor("bE", [NU, 1], F32, kind="ExternalInput")
    bD = nc.dram_tensor("bD", [COUT, 1], F32, kind="ExternalInput")
    out = nc.dram_tensor("out", [COUT, BL], F32, kind="ExternalOutput")

    with tile.TileContext(nc) as tc:
        with (
            tc.tile_pool(name="const", bufs=1) as cp,
            tc.tile_pool(name="xp", bufs=3) as xp,
            tc.tile_pool(name="qp", bufs=1) as qp,
            tc.tile_pool(name="gbp", bufs=1) as gbp,
            tc.tile_pool(name="gp", bufs=1) as gp,
            tc.tile_pool(name="op", bufs=1) as op,
            tc.tile_pool(name="psy", bufs=1, space="PSUM") as psy,
            tc.tile_pool(name="psg", bufs=1, space="PSUM") as psg,
            tc.tile_pool(name="pso", bufs=1, space="PSUM") as pso,
        ):
            # ---- constants ----
            wS_t = cp.tile([NU, NU], F16, tag="wS")
            nc.sync.dma_start(wS_t[:], wS[:])
            wW_t = cp.tile([NU, NU], F16, tag="wW")
            nc.sync.dma_start(wW_t[:], wW[:])
            wM_t = cp.tile([NU, NU], F16, tag="wM")
            nc.sync.dma_start(wM_t[:], wM[:])
            wI_t = cp.tile([NU, NU], F16, tag="wI")
            nc.sync.dma_start(wI_t[:], wI[:])
            wE_t = cp.tile([DIN, NU], F16, tag="wE")
            nc.sync.dma_start(wE_t[:], wE[:])
            wD_t = cp.tile([NU, COUT], F32, tag="wD")
            nc.sync.dma_start(wD_t[:], wD[:])
            bE_t = cp.tile([NU, 1], F32, tag="bE")
            nc.sync.dma_start(bE_t[:], bE[:])
            bD_t = cp.tile([COUT, 1], F32, tag="bD")
            nc.sync.dma_start(bD_t[:], bD[:])

            # ---- state buffers, explicitly rotated (Tile pools reuse the
            # most-recently-freed slot, which serializes the pipeline) ----
            psum_ys = [psy.tile([NU, BL], F32, tag=f"py{i}", name=f"py{i}") for i in range(4)]
            psum_us = [psg.tile([NU, BL], F32, tag=f"pu{i}", name=f"pu{i}") for i in range(3)]
            q_tiles = [qp.tile([NU, BL], F16, tag=f"q{i}", name=f"q{i}") for i in range(3)]
            ub_tiles = [gbp.tile([NU, BL], F16, tag=f"ub{i}", name=f"ub{i}") for i in range(3)]
            u_tiles = [gp.tile([NU, BL], F32, tag=f"u{i}", name=f"u{i}") for i in range(2)]
            for tl_ in u_tiles + ub_tiles[:2] + [q_tiles[0]]:
                nc.gpsimd.memset(tl_[:], 0.0)

            # ---- ACT table preload (tanh) on scratch ----
            scratch = cp.tile([NU, 1], F32, tag="scratch")
            nc.scalar.activation(scratch[:], bE_t[:], Tanh, bias=0.0)

            # ---- PE warmup: dummy MMs to lift HAM to 2.4 GHz during DMA ----
            # (the warm bank doubles as the output-projection psum bank)
            warm = pso.tile([NU, NU], F32)
            for _ in range(warmup_mms):
                mm(warm[:], wI_t[:], wI_t[:], start=True, stop=True)

            # ---- recurrence ----
            # z(rt) is emitted one iteration EARLY (before ACT(rt-1)) so the
            # scheduler does not chain it behind the latest tanh.
            RTOT = repeats * T

            def emit_z(rt, xt):
                j = (rt % T) % CHUNK
                mm(
                    psum_ys[rt % 4][:], wE_t[:], xt[:, j * BL : (j + 1) * BL],
                    start=True, stop=(rt == 0),
                )

            def fetch_chunk(rt):
                c = (rt % T) // CHUNK
                xt = xp.tile([DIN, CHUNK * BL], F16, tag="x", name=f"x_{rt}")
                lo = c * CHUNK * BL
                hi = min((c + 1) * CHUNK * BL, T * BL)
                nc.sync.dma_start(xt[:, : hi - lo], xw[:, lo:hi])
                return xt

            xt = fetch_chunk(0)
            emit_z(0, xt)
            for rt in range(RTOT):
                psum_y = psum_ys[rt % 4]
                psum_y1 = psum_ys[(rt + 1) % 4]
                psum_u = psum_us[(rt + 1) % 3]
                q_cur = q_tiles[rt % 3]
                q_next = q_tiles[(rt + 1) % 3]
                ub_cur = ub_tiles[rt % 3]
                ub_next = ub_tiles[(rt + 1) % 3]
                u_cur = u_tiles[rt % 2]
                u_next = u_tiles[(rt + 1) % 2]

                # z for the next step first (no dependencies at all)
                if rt + 1 < RTOT:
                    if ((rt + 1) % T) % CHUNK == 0:
                        xt = fetch_chunk(rt + 1)
                    emit_z(rt + 1, xt)
                if rt > 0:
                    # chain matmul strictly first among q-gated work
                    mm(psum_y[:], wW_t[:], q_cur[:], start=False, stop=True)
                    mm(psum_u[:], wI_t[:], q_cur[:], start=True, stop=False)
                    mm(psum_u[:], wS_t[:], q_cur[:], start=False, stop=False)
                    if rt + 1 < RTOT:
                        mm(psum_y1[:], wW_t[:], q_cur[:], start=False, stop=False)
                        mm(psum_y1[:], wM_t[:], q_cur[:], start=False, stop=False)
                    # ub-gated tail (ub is one full period stale -> never blocks)
                    mm(psum_u[:], wS_t[:], ub_cur[:], start=False, stop=True)
                    if rt + 1 < RTOT:
                        mm(psum_y1[:], wW_t[:], ub_cur[:], start=False, stop=False)
                        mm(psum_y1[:], wM_t[:], ub_cur[:], start=False, stop=False)

                if rt > 0:
                    nc.vector.tensor_add(ub_next[:], u_cur[:], psum_u[:])
                    nc.vector.tensor_add(u_next[:], u_cur[:], psum_u[:])
                for _ in range(n_fill):
                    mm(warm[:, :BL], wI_t[:], wI_t[:, :BL],
                       start=True, stop=True)

                nc.scalar.activation(q_next[:], psum_y[:], Tanh, bias=bE_t[:])

            # ---- epilogue: g_T = u_T + q_T; project ----
            RT = repeats * T
            q_fin = q_tiles[RT % 3]
            u_fin = u_tiles[RT % 2]
            g_fin = gp.tile([NU, BL], F32, tag="gfin")
            nc.vector.tensor_add(g_fin[:], u_fin[:], q_fin[:])

            psum_o = warm[:COUT, :BL]
            mm(psum_o, wD_t[:], g_fin[:], start=True, stop=True)
            o_t = op.tile([COUT, BL], F32)
            nc.scalar.add(o_t[:], psum_o, bD_t[:])
            nc.sync.dma_start(out[:], o_t[:])

    nc.compile()
    return nc


def host_prep(x, E_w, E_b, B_p, C_p, D_w, D_b, T=None):
    """Compute derived matrices + per-core shards. Returns in_maps list."""
    if T is None:
        T = x.shape[1]
    I = np.eye(NU, dtype=np.float64)
    B_p = B_p.astype(np.float64)
    C_p = C_p.astype(np.float64)
    A = BETA * (B_p - B_p.T) + (1.0 - BETA) * (B_p + B_p.T) - GAMMA_A * I
    W = BETA * (C_p - C_p.T) + (1.0 - BETA) * (C_p + C_p.T) - GAMMA_W * I
    S = (EPS * ALPHA) * A
    Wp = EPS * W
    SWp = S @ Wp
    Dw = EPS * D_w.astype(np.float64)

    wS = S.astype(np.float16)
    wW = Wp.astype(np.float16)
    wM = SWp.astype(np.float16)
    wI = np.eye(NU, dtype=np.float16)
    wE = E_w.T.astype(np.float16)                      # [DIN, NU] lhsT
    wD = Dw.T.astype(np.float32)                       # [NU, COUT] lhsT
    bE = E_b.reshape(NU, 1).astype(np.float32)
    bD = D_b.reshape(COUT, 1).astype(np.float32)

    nb = x.shape[0] // BL
    in_maps = []
    for i in range(nb):
        xc = x[i * BL : (i + 1) * BL, :T, :]           # [BL, T, DIN]
        xpre = np.ascontiguousarray(
            xc.transpose(2, 1, 0).reshape(DIN, T * BL)
        ).astype(np.float16)                           # [DIN, T*BL], col = t*BL + b
        in_maps.append(
            dict(xw=xpre, wS=wS, wW=wW, wM=wM, wI=wI, wE=wE, wD=wD, bE=bE, bD=bD)
        )
    return in_maps


def assemble_out(results):
    """results: list of per-core dicts with out [COUT, BL] -> [B, COUT]."""
    return np.concatenate([r["out"].T for r in results], axis=0).astype(np.float32)


def kernel(x, E_w, E_b, B_p, C_p, D_w, D_b):
    x = np.asarray(x, dtype=np.float32)
    E_w = np.asarray(E_w, dtype=np.float32)
    E_b = np.asarray(E_b, dtype=np.float32)
    B_p = np.asarray(B_p, dtype=np.float32)
    C_p = np.asarray(C_p, dtype=np.float32)
    D_w = np.asarray(D_w, dtype=np.float32)
    D_b = np.asarray(D_b, dtype=np.float32)
    nc = build_rnn(T_FULL)
    in_maps = host_prep(x, E_w, E_b, B_p, C_p, D_w, D_b, T=T_FULL)
    res = run_bass_kernel_spmd(nc, in_maps, core_ids=list(range(NCORES)))
    return assemble_out(res.results)


if __name__ == "__main__":
    rng = np.random.default_rng(0)
    d = np.load("cache_io.npz")
    out = kernel(d["x"], d["E_w"], d["E_b"], d["B_p"], d["C_p"], d["D_w"], d["D_b"])
    exp = d["expected"]
    rel = np.linalg.norm(out - exp) / np.linalg.norm(exp)
    print("rel err:", rel)

